# revision 1
# baseline (speedup 1.0000x reference)
"""Trainium2 Bass kernel for nn_ComplexEMA.

Math: reference computes, per (batch b, channel d), a causal convolution of
x[b,d,:] with a kernel k_d built from N=16 decaying complex exponentials
(radius <= 0.858 for this model's parameters), plus a residual omega_d*x.
radius^256 < 1e-17, so k_d is exactly representable by its first 256 taps.

Algorithm: chunk L=2048 into 16 chunks of C=128.  Then
    y[b,d,c*C+t] = sum_t' x[b,d,c*C+t']   * M0_d[t',t]
                 + sum_t' x[b,d,(c-1)C+t']* M1_d[t',t]
with M0_d[t',t] = k_d[t-t'] (+ omega_d on the diagonal), M1_d[t',t] =
k_d[C+t-t'].  Each channel is two 128x128 fp16 matmuls with the x-chunks as
the stationary operand ([t', (c,b)] layout) so the PSUM output lands in
[(c,b), t] layout, which DMAs out as contiguous 256B rows.

Sharding: channels D=1024 split across the 8 cores (128 each); batch stays
whole so every matmul has 128 moving columns.
"""

import math

import numpy as np

B, D, L, N = 8, 1024, 2048, 16
NCORES = 8
DLOC = D // NCORES          # 128 channels per core
C = 128                     # chunk size == significant taps per block
NCH = L // C                # 16 chunks
G = 16                      # channels per pipelined group
NGROUPS = DLOC // G
PADB = 8                    # zero columns ahead of each channel block
BLK = C + PADB              # 136 columns per channel block in SBUF
NBLK = G + 1                # extra block so the strided pad memset is one AP

_NC_CACHE = {}
LAST_EXEC_NS = None
LAST_RESULTS = None


def _host_weights(alpha, delta, theta, gamma_real, gamma_imag, omega):
    """Exact (float64) first 256 taps of the per-channel kernel, packed as
    W[d, t', m, t]: m=0 -> M0 (with omega on the diagonal), m=1 -> M1."""
    sig = lambda v: 1.0 / (1.0 + np.exp(-v.astype(np.float64)))
    th = sig(theta[:, 0, 0]) * (2.0 * math.pi / N)            # (D,)
    wav = np.arange(1, N + 1, dtype=np.float64)
    phi = wav[None, :] * th[:, None]                          # (D,N)
    a = sig(alpha[:, :, 0])
    d_ = sig(delta[:, :, 0])
    radius = np.minimum(1.0 - a * d_, 1.0)
    gp = (gamma_real.astype(np.float64) + 1j * gamma_imag.astype(np.float64))
    gp *= math.sqrt(1.0 / N) * a
    q = radius * np.exp(1j * phi)                             # (D,N)

    taps = np.arange(2 * C, dtype=np.float64)
    ql = q[:, :, None] ** taps[None, None, :]                 # (D,N,2C)
    k = np.real((gp[:, :, None] * ql).sum(1))                 # (D,2C)
    k0 = k[:, :C].copy()
    k0[:, 0] += omega.astype(np.float64)

    t = np.arange(C)
    lag0 = t[None, :] - t[:, None]                            # (t',t)
    m0 = np.where(lag0 >= 0, k0[:, np.clip(lag0, 0, C - 1)], 0.0)
    m1 = k[:, C + lag0]                                       # lags 1..255
    w = np.stack([m0, m1], axis=2)                            # (D, t', m, t)
    return np.ascontiguousarray(w.astype(np.float16))


def _build_nc():
    import concourse.bass as bass  # noqa: F401
    import concourse.mybir as mybir
    import concourse.tile as tile
    from concourse import bacc

    f16 = mybir.dt.float16
    f32 = mybir.dt.float32

    nc = bacc.Bacc(None, target_bir_lowering=False)
    xt = nc.declare_dram_parameter("xt", [DLOC, C, BLK], f16, isOutput=False)
    w = nc.declare_dram_parameter("w", [DLOC, C, 2, C], f16, isOutput=False)
    y = nc.declare_dram_parameter("y", [NGROUPS, NCH * B, G, C], f16, isOutput=True)

    with tile.TileContext(nc) as tc:
        with (
            tc.tile_pool(name="xp", bufs=3) as xp,
            tc.tile_pool(name="wp", bufs=3) as wp,
            tc.tile_pool(name="yp", bufs=3) as yp,
            tc.tile_pool(name="ps", bufs=8, space="PSUM") as ps,
        ):
            for g in range(NGROUPS):
                g0 = g * G
                xg = xp.tile([C, G * BLK], f16)
                nc.sync.dma_start(
                    out=xg[:].rearrange("p (ch q) -> p ch q", q=BLK),
                    in_=xt[g0 : g0 + G].rearrange("ch t q -> t ch q"),
                )
                wg = wp.tile([C, G * 2 * C], f16)
                nc.sync.dma_start(
                    out=wg[:].rearrange("p (ch m u) -> p ch m u", m=2, u=C),
                    in_=w[g0 : g0 + G].rearrange("ch t m u -> t ch m u"),
                )
                ysb = yp.tile([C, G * C], f16)
                for ch in range(G):
                    base = ch * BLK
                    # One PSUM tile per channel: start=True clears the whole
                    # bank, so accumulation groups must not share banks.
                    yps = ps.tile([C, C], f32, tag="yps")
                    nc.tensor.matmul(
                        yps[:],
                        lhsT=xg[:, base + PADB : base + PADB + C],
                        rhs=wg[:, ch * 2 * C : ch * 2 * C + C],
                        start=True,
                        stop=False,
                    )
                    nc.tensor.matmul(
                        yps[:],
                        lhsT=xg[:, base : base + C],
                        rhs=wg[:, ch * 2 * C + C : ch * 2 * C + 2 * C],
                        start=False,
                        stop=True,
                    )
                    sl = slice(ch * C, (ch + 1) * C)
                    if ch % 2 == 0:
                        nc.vector.tensor_copy(ysb[:, sl], yps[:])
                    else:
                        nc.scalar.copy(ysb[:, sl], yps[:])
                nc.sync.dma_start(
                    out=y[g],
                    in_=ysb[:].rearrange("p (ch u) -> p ch u", u=C),
                )
    nc.compile()
    return nc


def _get_nc():
    if "nc" not in _NC_CACHE:
        _NC_CACHE["nc"] = _build_nc()
    return _NC_CACHE["nc"]


def kernel(x, alpha, delta, theta, gamma_real, gamma_imag, omega, **_):
    global LAST_EXEC_NS, LAST_RESULTS
    import os

    from concourse.bass_utils import run_bass_kernel_spmd

    x = np.asarray(x)
    wfull = _host_weights(
        np.asarray(alpha), np.asarray(delta), np.asarray(theta),
        np.asarray(gamma_real), np.asarray(gamma_imag), np.asarray(omega),
    )
    # x[b,d,c*C+t'] -> xt[d, t', pad8 + (c,b)] so each channel DMAs as 272B
    # rows whose leading 8 columns are the zeros the M1 matmul view needs.
    xtf = np.zeros((D, C, BLK), dtype=np.float16)
    xtf[:, :, PADB:] = (
        x.reshape(B, D, NCH, C).transpose(1, 3, 2, 0).reshape(D, C, NCH * B)
    )

    nc = _get_nc()
    in_maps = [
        {"xt": xtf[i * DLOC : (i + 1) * DLOC], "w": wfull[i * DLOC : (i + 1) * DLOC]}
        for i in range(NCORES)
    ]
    trace = bool(int(os.environ.get("KERNEL_TRACE", "0")))
    res = run_bass_kernel_spmd(nc, in_maps, list(range(NCORES)), trace=trace)
    LAST_EXEC_NS = res.exec_time_ns
    LAST_RESULTS = res

    y = np.empty((B, D, L), dtype=np.float32)
    for i in range(NCORES):
        yi = res.results[i]["y"]                 # [NGROUPS, (c,b), G, C] fp16
        yi = yi.reshape(NGROUPS, NCH, B, G, C).transpose(2, 0, 3, 1, 4)
        y[:, i * DLOC : (i + 1) * DLOC, :] = (
            yi.reshape(B, DLOC, L).astype(np.float32)
        )
    return y

